# revision 29
# baseline (speedup 1.0000x reference)
"""Trainium2 distributed kernel for ALRDLinear + 3-bit per-tensor fake-quant.

Reference computation (tokens=8192, in=4096, rank=1024, out=4096, f32):
    y   = input @ B_w.T                       # [tokens, rank]
    y_q = fake_quant(y)                       # per-tensor symmetric 3-bit
    out = y_q @ A_w.T + A_b                   # [tokens, out]

Distribution: data-parallel over tokens across 8 NeuronCores (1024 tok/core).
Weights replicated. The only cross-core dependency is the per-tensor amax,
exchanged with one 64-byte AllGather and reduced locally.

Numerics: matmul1 runs as 3 accumulating fp16 matmul passes on hi/lo splits
(x = xh + xl, B = Bh + Bl; the lo*lo term is dropped) giving ~1e-6 abs error
in y. That precision is required: y feeds round(y/scale), and rounding-boundary
flips are amplified by the 3-bit step size (bf16 or fp32r matmuls fail the
2e-2 gate through this amplification). Quantization uses the +1.5*2^23 RNE
trick with no clip (|round(y/scale)| <= 3 holds by construction of scale);
y_q = q*scale is exact-int times scale stored bf16. Matmul2 runs y_q (bf16)
against bf16 A-weights, bias added on eviction.

Perf notes (measured on TRN2, 8 cores): PE issues N=512 fp16/bf16 matmuls
every ~263ns here; a NEFF containing any collective_compute gets its PE
clock capped ~2.08GHz from load (vs 2.4GHz without — measured, presence-
based), which is why the amax exchange is kept to a single tiny AllGather.
All cross-core DMA payloads are single-descriptor contiguous rows: a
[128,1] partition-strided bounce costs ~7.5us in 4-byte descriptors.
"""

import numpy as np
import ml_dtypes

P = 128
TOK, IN_F, OUT_F, RANK = 8192, 4096, 4096, 1024
NCORES = 8
TPC = TOK // NCORES            # tokens per core
KT1 = IN_F // P                # 32 contraction tiles for matmul1
MR = RANK // P                 # 8 rank tiles
NT1 = TPC // 512               # 2 token column-tiles in matmul1
MT2 = TPC // P                 # 8 token row-tiles in matmul2
NT2 = OUT_F // 512             # 8 out-feature tiles

QMAX = 3.0
QMIN = -4.0
MAGIC = 1.5 * 2.0**23          # round-to-nearest-even integer trick

_CACHE = {}


def _build():
    import concourse.mybir as mybir
    import concourse.tile as tile
    from concourse import bacc
    from concourse import bass_isa

    nc = bacc.Bacc(None, target_bir_lowering=False, debug=False, num_devices=NCORES)
    f32, f16, bf16 = mybir.dt.float32, mybir.dt.float16, mybir.dt.bfloat16

    xh_d = nc.dram_tensor("xh", [P, NT1, KT1, 512], f16, kind="ExternalInput")
    xl_d = nc.dram_tensor("xl", [P, NT1, KT1, 512], f16, kind="ExternalInput")
    bh_d = nc.dram_tensor("bh", [P, MR, KT1, P], f16, kind="ExternalInput")
    bl_d = nc.dram_tensor("bl", [P, MR, KT1, P], f16, kind="ExternalInput")
    aw_d = nc.dram_tensor("aw", [P, NT2, MR, 512], bf16, kind="ExternalInput")
    bias_d = nc.dram_tensor("bias", [P, OUT_F], f32, kind="ExternalInput")
    out_d = nc.dram_tensor("out", [TPC, OUT_F], f32, kind="ExternalOutput")

    cc_in = nc.dram_tensor("cc_in", [1, 16], f32)
    cc_out = nc.dram_tensor("cc_out", [NCORES, 16], f32, addr_space="Shared")

    ts = lambda i, s: slice(i * s, (i + 1) * s)

    with tile.TileContext(nc) as tc:
        with (
            tc.tile_pool(name="stats", bufs=1) as stats,
            tc.tile_pool(name="ypool", bufs=1) as ypool,
            tc.tile_pool(name="psum", bufs=8, space="PSUM") as psum,
        ):
            y_t = ypool.tile([P, MR, TPC], f32, tag="y")
            am_part = stats.tile([P, MR * NT1], f32, tag="am_part")
            am1 = stats.tile([P, 1], f32, tag="am1")
            am_b = stats.tile([P, 1], f32, tag="am_b")

            # ---------------- phase 1: y.T = B @ x.T (fp16 3-pass) -------
            with (
                tc.tile_pool(name="xpool", bufs=1) as xpool,
                tc.tile_pool(name="bpool", bufs=2) as bpool,
            ):
                xh_t = xpool.tile([P, NT1, KT1, 512], f16, tag="xh")
                xl_t = xpool.tile([P, NT1, KT1, 512], f16, tag="xl")
                # DMAs in consumption order, in chunks with >=4KB contiguous
                # runs per partition (1KB segments starve the stream).
                KG = 4
                bh_t = bpool.tile([P, KT1, P], f16, tag="bh")
                bl_t = bpool.tile([P, KT1, P], f16, tag="bl")
                for g in range(KT1 // KG):
                    sl = ts(g, KG)
                    nc.sync.dma_start(bh_t[:, sl], bh_d[:, 0, sl])
                    nc.sync.dma_start(bl_t[:, sl], bl_d[:, 0, sl])
                    nc.sync.dma_start(xh_t[:, 0, sl], xh_d[:, 0, sl])
                    nc.sync.dma_start(xl_t[:, 0, sl], xl_d[:, 0, sl])
                for g in range(KT1 // KG):
                    nc.sync.dma_start(
                        xh_t[:, 1, ts(g, KG)], xh_d[:, 1, ts(g, KG)])
                    nc.sync.dma_start(
                        xl_t[:, 1, ts(g, KG)], xl_d[:, 1, ts(g, KG)])

                for mr in range(MR):
                    if mr > 0:
                        bh_t = bpool.tile([P, KT1, P], f16, tag="bh")
                        bl_t = bpool.tile([P, KT1, P], f16, tag="bl")
                        nc.sync.dma_start(bh_t[:], bh_d[:, mr])
                        nc.sync.dma_start(bl_t[:], bl_d[:, mr])
                    for nt in range(NT1):
                        ps = psum.tile([P, 512], f32, tag="ps")
                        for k in range(KT1):
                            nc.tensor.matmul(
                                ps[:], bh_t[:, k], xh_t[:, nt, k],
                                start=(k == 0), stop=False)
                            nc.tensor.matmul(
                                ps[:], bh_t[:, k], xl_t[:, nt, k],
                                start=False, stop=False)
                            nc.tensor.matmul(
                                ps[:], bl_t[:, k], xh_t[:, nt, k],
                                start=False, stop=(k == KT1 - 1))
                        idx = mr * NT1 + nt
                        nc.vector.tensor_reduce(
                            am_part[:, idx : idx + 1], ps[:],
                            axis=mybir.AxisListType.X, op=mybir.AluOpType.max,
                            apply_absolute_value=True)
                        nc.scalar.copy(y_t[:, mr, ts(nt, 512)], ps[:])

            # ---------------- amax all-gather + scale ---------------------
            # Keep every cross-core transfer a single contiguous descriptor
            # (a [128,1] partition-strided DMA costs ~7.5us in 4B descriptors).
            nc.vector.tensor_reduce(
                am1[:], am_part[:], axis=mybir.AxisListType.X,
                op=mybir.AluOpType.max)
            nc.gpsimd.partition_all_reduce(
                am_b[:], am1[:], channels=P, reduce_op=bass_isa.ReduceOp.max)
            row16 = stats.tile([1, 16], f32, tag="row16")
            nc.vector.tensor_copy(row16[0:1, :], am_b[0:1, 0:1].to_broadcast([1, 16]))
            nc.sync.dma_start(cc_in[:, :], row16[0:1, :])
            nc.gpsimd.collective_compute(
                "AllGather", mybir.AluOpType.bypass,
                replica_groups=[list(range(NCORES))],
                ins=[cc_in.ap().opt()], outs=[cc_out.ap().opt()])
            amrow = stats.tile([1, NCORES * 16], f32, tag="amrow")
            ret_dma = nc.sync.dma_start(
                amrow[0:1, :], cc_out[:, :].rearrange("c x -> (c x)")[None, :])
            amg1 = stats.tile([1, 1], f32, tag="amg1")
            nc.vector.tensor_reduce(
                amg1[0:1, :], amrow[0:1, :], axis=mybir.AxisListType.X,
                op=mybir.AluOpType.max)
            # si = [scale, 1/scale] on partition 0; Q7-broadcast to all 128
            si = stats.tile([1, 2], f32, tag="si")
            nc.vector.tensor_scalar(
                si[0:1, 0:1], amg1[0:1, :], 1e-8, float(np.float32(1.0 / QMAX)),
                mybir.AluOpType.max, mybir.AluOpType.mult)
            nc.vector.reciprocal(si[0:1, 1:2], si[0:1, 0:1])
            bc = stats.tile([P, 2], f32, tag="bc")
            nc.gpsimd.partition_broadcast(bc[:], si[0:1, :], channels=P)
            scale_t = bc[:, 0:1]
            inv_t = bc[:, 1:2]

            # ---------------- phase 2: quant + out = q @ (Aw*scale) + b --
            with (
                tc.tile_pool(name="qpool", bufs=1) as qpool,
                tc.tile_pool(name="tpool", bufs=1) as tpool,
                tc.tile_pool(name="apool", bufs=2) as apool,
                tc.tile_pool(name="opool", bufs=4) as opool,
                tc.tile_pool(name="biasp", bufs=1) as biasp,
            ):
                from concourse.tile_rust import add_dep_helper

                # Phase-2 bulk loads would otherwise be released exactly at
                # mm1-end (their SBUF overlaps the freed x pool) and their
                # queue drain delays the tiny amax bounce DMA by ~10us. Gate
                # them behind the collective's return DMA instead.
                bias_t = biasp.tile([P, OUT_F], f32, tag="bias")
                bias_dma = nc.sync.dma_start(bias_t[:], bias_d[:, :])

                # No explicit clip needed: scale = amax/QMAX with amax taken
                # over these same y values, so |round(y*inv)| <= QMAX always
                # (and QMIN=-4 < -QMAX is unreachable for symmetric data).
                q_t = qpool.tile([P, MR, TPC], bf16, tag="q")
                for mt in range(MT2):
                    sl = ts(mt, P)
                    t1 = tpool.tile([P, MR, P], f32, tag="t1")
                    # t1 = y*inv + MAGIC  (RNE to integer in the f32 lattice)
                    nc.vector.tensor_scalar(
                        t1[:], y_t[:, :, sl], inv_t[:], MAGIC,
                        mybir.AluOpType.mult, mybir.AluOpType.add)
                    # y_q = (t1 - MAGIC) * scale -> bf16 (8 exact levels)
                    nc.vector.tensor_scalar(
                        q_t[:, :, sl], t1[:], -MAGIC, scale_t[:],
                        mybir.AluOpType.add, mybir.AluOpType.mult)

                last_a0 = None
                for nt in range(NT2):
                    a_t = apool.tile([P, MR, 512], bf16, tag="aw")
                    if nt == 0:
                        # kr-chunked so mm2 can start right after quant blk 0;
                        # bias and the nt=1 slice queue strictly behind it.
                        for c in range(4):
                            a_dma = nc.sync.dma_start(
                                a_t[:, ts(c, 2)], aw_d[:, nt, ts(c, 2)])
                            add_dep_helper(a_dma.ins, ret_dma.ins,
                                           reason="after amax handshake")
                            last_a0 = a_dma
                        add_dep_helper(bias_dma.ins, last_a0.ins,
                                       reason="a0 slice has DMA priority")
                    else:
                        a_dma = nc.sync.dma_start(a_t[:], aw_d[:, nt])
                        if nt == 1:
                            add_dep_helper(a_dma.ins, bias_dma.ins,
                                           reason="a0+bias first")
                    for mt in range(MT2):
                        ps2 = psum.tile([P, 512], f32, tag="ps")
                        for kr in range(MR):
                            nc.tensor.matmul(
                                ps2[:], q_t[:, kr, ts(mt, P)], a_t[:, kr],
                                start=(kr == 0), stop=(kr == MR - 1))
                        o_t = opool.tile([P, 512], f32, tag="o")
                        nc.vector.tensor_tensor(
                            o_t[:], ps2[:], bias_t[:, ts(nt, 512)],
                            mybir.AluOpType.add)
                        nc.sync.dma_start(out_d[ts(mt, P), ts(nt, 512)], o_t[:])

    nc.compile()
    return nc


def _get_nc():
    if "nc" not in _CACHE:
        _CACHE["nc"] = _build()
    return _CACHE["nc"]


def kernel(input, B_w, A_w, A_b):
    from concourse import bass_utils

    nc = _get_nc()

    f32 = np.float32
    bf16 = ml_dtypes.bfloat16

    # Weights (replicated, pre-laid-out for the PE's [K-on-partitions] form).
    BwT = np.ascontiguousarray(B_w.astype(f32, copy=False).T)     # [IN_F, RANK]
    Bh = BwT.astype(np.float16)
    Bl = (BwT - Bh.astype(f32)).astype(np.float16)
    Bh = np.ascontiguousarray(Bh.reshape(KT1, P, MR, P).transpose(1, 2, 0, 3))
    Bl = np.ascontiguousarray(Bl.reshape(KT1, P, MR, P).transpose(1, 2, 0, 3))

    AwT = np.ascontiguousarray(A_w.astype(f32, copy=False).T)     # [RANK, OUT_F]
    Aw = np.ascontiguousarray(
        AwT.astype(bf16).reshape(MR, P, NT2, 512).transpose(1, 2, 0, 3))

    bias_rep = np.ascontiguousarray(
        np.broadcast_to(A_b.astype(f32, copy=False), (P, OUT_F)))

    in_maps = []
    for c in range(NCORES):
        xT = np.ascontiguousarray(input[c * TPC : (c + 1) * TPC].astype(f32, copy=False).T)
        xh = xT.astype(np.float16)
        xl = (xT - xh.astype(f32)).astype(np.float16)
        # [IN_F, TPC] -> [P, NT1, KT1, 512]: per-partition contiguous chunks
        xh = np.ascontiguousarray(
            xh.reshape(KT1, P, NT1, 512).transpose(1, 2, 0, 3))
        xl = np.ascontiguousarray(
            xl.reshape(KT1, P, NT1, 512).transpose(1, 2, 0, 3))
        in_maps.append(
            {"xh": xh, "xl": xl, "bh": Bh, "bl": Bl, "aw": Aw, "bias": bias_rep}
        )

    res = bass_utils.run_bass_kernel_spmd(nc, in_maps, core_ids=list(range(NCORES)))
    out = np.concatenate([res.results[c]["out"] for c in range(NCORES)], axis=0)
    return out.astype(np.float32, copy=False)
